# revision 1
# baseline (speedup 1.0000x reference)
"""Trainium2 Bass kernel for nn_BDH_4406636445711 (dense transformer).

Sharding: 8 cores = data-parallel over B(2) x tensor-parallel over H(4).
Core c handles (b = c//4, h = c%4): its head's Dx/Dy slices, E rows, and a
V/4 shard of the readout. Per layer the y@E partial is AllReduced within
each b-group of 4 cores; v stays replicated inside the group. The host
stitches the 8 per-core [VS, T] logit shards into the full [B, T, V].

Matmuls run in float32r (full-rate fp32 with 11-bit RNE mantissa rounding,
~1.2e-4 relative noise vs fp32's 4x slower exact mode). v is kept
transposed ([D, T] "dT layout") as the primary representation; LayerNorms
over D (the partition dim) use ones-vector matmul column sums plus PE
rank-1 broadcast of the per-token -mean/rstd back to [128, T] tiles.
"""

import os
import sys

sys.path.insert(0, "/opt/trn_rl_repo")

import numpy as np

import concourse.bass as bass
import concourse.tile as tile
from concourse import bacc, mybir
from concourse.bass_utils import run_bass_kernel_spmd
from concourse.masks import make_identity
from concourse import library_config

F32 = mybir.dt.float32
F32R = mybir.dt.float32r
I32 = mybir.dt.int32
AF = mybir.ActivationFunctionType
OP = mybir.AluOpType

B, T, H, D, K, V, L = 2, 2048, 4, 256, 1024, 32000, 6
VS = V // 4          # vocab shard per core within a b-group
EPS = 1e-5
NT = T // 128        # 16 token tiles
NKT = K // 128       # 8 k' tiles
ND = D // 128        # 2 d tiles
TH = T // 2          # t-half = 1024
NS = TH // 512       # 512-wide matmul chunks per t-half

N_LAYERS = int(os.environ.get("KRN_LAYERS", str(L)))
DO_READOUT = os.environ.get("KRN_READOUT", "1") == "1"


def build(nc):
    # ---- DRAM parameters (per core) ----
    tok_d = nc.dram_tensor("tok", [T], I32, kind="ExternalInput")
    emb_d = nc.dram_tensor("emb", [V, D], F32, kind="ExternalInput")
    posT_d = nc.dram_tensor("posT", [D, T], F32, kind="ExternalInput")
    dx_d = nc.dram_tensor("dx", [D, K], F32R, kind="ExternalInput")
    dy_d = nc.dram_tensor("dy", [D, K], F32R, kind="ExternalInput")
    e_d = nc.dram_tensor("eh", [K, D], F32R, kind="ExternalInput")
    ro_d = nc.dram_tensor("ro", [D, VS], F32R, kind="ExternalInput")
    cos_d = nc.dram_tensor("cosh", [4, 128, T], F32, kind="ExternalInput")
    sin_d = nc.dram_tensor("sinh", [4, 128, T], F32, kind="ExternalInput")
    out_d = nc.dram_tensor("logitsT", [VS, T], F32, kind="ExternalOutput")
    vdbg_d = nc.dram_tensor("vdbg", [ND, 128, T], F32, kind="ExternalOutput")

    groups = [[0, 1, 2, 3], [4, 5, 6, 7]]

    with tile.TileContext(nc) as tc:
        with (
            nc.allow_low_precision(reason="float32r rounding is intentional"),
            tc.tile_pool(name="persist", bufs=1) as pp,
            tc.tile_pool(name="w8", bufs=6) as w8p,
            tc.tile_pool(name="t4", bufs=6) as t4p,
            tc.tile_pool(name="stats", bufs=3) as stp,
            tc.tile_pool(name="psmm", bufs=2, space="PSUM") as psmm,
            tc.tile_pool(name="psacc", bufs=1, space="PSUM") as psacc,
            tc.tile_pool(name="dram", bufs=1, space="DRAM") as dpool,
        ):
            _ctr = [0]

            def _nm(p):
                _ctr[0] += 1
                return f"{p}{_ctr[0]}"

            def w8(dt=F32):
                return w8p.tile([128, T], dt, tag="w8", name=_nm("w8_"))

            def w8n(dt=F32):
                return w8p.tile([128, ND, TH], dt, tag="w8", name=_nm("w8n_"))

            def t4(dt=F32):
                return t4p.tile([128, TH], dt, tag="t4", name=_nm("t4_"))

            def pmm(shape=None, dt=F32):
                return psmm.tile(shape or [128, TH], dt, tag="mm", name=_nm("mm_"))

            # ---- constants ----
            ident_f = t4p.tile([128, 128], F32, tag="t4", name="identf")
            make_identity(nc, ident_f[:])
            ident_r = pp.tile([128, 128], F32R)
            nc.vector.tensor_copy(ident_r[:], ident_f[:])
            ones_pf = pp.tile([128, 1], F32)
            nc.vector.memset(ones_pf[:], 1.0)
            ones_p = pp.tile([128, 1], F32R)
            nc.vector.tensor_copy(ones_p[:], ones_pf[:])
            ones_cf = pp.tile([1, 128], F32)
            nc.vector.memset(ones_cf[:], 1.0)
            ones_c = pp.tile([1, 128], F32R)
            nc.vector.tensor_copy(ones_c[:], ones_cf[:])
            eps_p = pp.tile([128, 1], F32)
            nc.vector.memset(eps_p[:], EPS)
            eps_1 = pp.tile([1, 1], F32)
            nc.vector.memset(eps_1[:], EPS)
            nc.gpsimd.load_library(library_config.attn)

            # ---- persistent tensors ----
            vT = pp.tile([128, ND, T], F32R)          # v (dT layout), updated in place
            qT = pp.tile([128, NKT, T], F32R)
            vp_td = pp.tile([128, NT, D], F32R)       # (v+pos) in td layout
            dx_sb = pp.tile([128, ND, K], F32R)
            nc.sync.dma_start(dx_sb[:], dx_d.ap().rearrange("(c p) k -> p c k", p=128))
            dy_sb = pp.tile([128, ND, K], F32R)
            nc.sync.dma_start(dy_sb[:], dy_d.ap().rearrange("(c p) k -> p c k", p=128))
            e_sb = pp.tile([128, NKT, D], F32R)
            nc.sync.dma_start(e_sb[:], e_d.ap().rearrange("(c p) d -> p c d", p=128))

            # ---- internal DRAM ----
            xspill = dpool.tile([NKT, 128, T], F32, tag="xspill")
            cc_in = [dpool.tile([ND, 128, TH], F32, tag=f"cci{i}", name=f"cci{i}")
                     for i in range(2)]
            cc_out = [dpool.tile([ND, 128, TH], F32, tag=f"cco{i}", name=f"cco{i}")
                      for i in range(2)]

            def pstats(negmean_src_ps, s2_src_ps, n):
                """negmean=-s1/n, rstd=1/sqrt(s2/n-mean^2+eps) as [1, TH] f32r."""
                negmean = stp.tile([1, TH], F32R, tag="st", name=_nm("st_"))
                nc.vector.tensor_scalar_mul(negmean[:], negmean_src_ps, -1.0 / n)
                m2 = stp.tile([1, TH], F32, tag="st", name=_nm("st_"))
                nc.vector.tensor_mul(m2[:], negmean[:].bitcast(F32),
                                     negmean[:].bitcast(F32))
                var = stp.tile([1, TH], F32, tag="st", name=_nm("st_"))
                nc.vector.scalar_tensor_tensor(
                    out=var[:], in0=s2_src_ps, scalar=1.0 / n, in1=m2[:],
                    op0=OP.mult, op1=OP.subtract)
                lnv = stp.tile([1, TH], F32, tag="st", name=_nm("st_"))
                nc.scalar.activation(lnv[:], var[:], AF.Ln, bias=eps_1[:])
                rstd = stp.tile([1, TH], F32R, tag="st", name=_nm("st_"))
                nc.scalar.activation(rstd[:], lnv[:], AF.Exp, scale=-0.5)
                return negmean, rstd

            def colsums(src, t0):
                """s1[t]=sum_d src[d,t], s2[t]=sum_d src[d,t]^2 over ND tiles.

                src is a [128, ND, T]-like f32r AP ([d-part, dc, t]); returns
                two [1, TH] psum tiles for the t-half starting at t0."""
                s1 = pmm()
                for dc in range(ND):
                    for ns in range(NS):
                        nc.tensor.matmul(
                            s1[:1, ns * 512:(ns + 1) * 512], ones_p[:],
                            src[:, dc, t0 + ns * 512:t0 + (ns + 1) * 512],
                            start=(dc == 0), stop=(dc == ND - 1),
                            skip_group_check=True)
                sq = w8n(F32R)
                for dc in range(ND):
                    nc.scalar.activation(sq[:, dc],
                                         src[:, dc, t0:t0 + TH].bitcast(F32),
                                         AF.Square)
                s2 = pmm()
                for dc in range(ND):
                    for ns in range(NS):
                        nc.tensor.matmul(
                            s2[:1, ns * 512:(ns + 1) * 512], ones_p[:],
                            sq[:, dc, ns * 512:(ns + 1) * 512],
                            start=(dc == 0), stop=(dc == ND - 1),
                            skip_group_check=True)
                return s1[:1, :], s2[:1, :]

            def bcast(vec):
                """PE rank-1 broadcast of a [1, TH] f32r vector to [128, TH] psum."""
                out = pmm()
                for ns in range(NS):
                    nc.tensor.matmul(out[:, ns * 512:(ns + 1) * 512], ones_c[:],
                                     vec[:, ns * 512:(ns + 1) * 512],
                                     start=True, stop=True)
                return out

            # ============ embedding gather + LN -> v0 -> transpose to vT ============
            idx = pp.tile([128, NT], I32)
            nc.sync.dma_start(idx[:], tok_d.ap().rearrange("(n p) -> p n", p=128))
            for n in range(NT):
                gat = t4p.tile([128, D], F32, tag="t4")
                nc.gpsimd.indirect_dma_start(
                    out=gat[:], out_offset=None, in_=emb_d.ap(),
                    in_offset=bass.IndirectOffsetOnAxis(ap=idx[:, n:n + 1], axis=0),
                )
                stats = t4p.tile([128, 6], F32, tag="t4")
                nc.vector.bn_stats(out=stats[:], in_=gat[:])
                mv = t4p.tile([128, 2], F32, tag="t4")
                nc.vector.bn_aggr(out=mv[:], in_=stats[:])
                std = t4p.tile([128, 1], F32, tag="t4")
                nc.scalar.activation(std[:], mv[:, 1:2], AF.Sqrt, bias=eps_p[:])
                rstd = t4p.tile([128, 1], F32, tag="t4")
                nc.vector.reciprocal(rstd[:], std[:])
                v0 = t4p.tile([128, D], F32R, tag="t4")
                nc.vector.tensor_scalar(
                    out=v0[:], in0=gat[:], scalar1=mv[:, 0:1], scalar2=rstd[:],
                    op0=OP.subtract, op1=OP.mult)
                for dc in range(ND):
                    tp = pmm([128, 128], F32R)
                    nc.tensor.transpose(out=tp[:], in_=v0[:, dc * 128:(dc + 1) * 128],
                                        identity=ident_r[:])
                    nc.vector.tensor_copy(vT[:, dc, n * 128:(n + 1) * 128], tp[:])

            def phaseA(th):
                """v[:, th-half] += pos; transpose that half into vp_td."""
                for dc in range(ND):
                    pch = t4()
                    nc.sync.dma_start(
                        pch[:], posT_d.ap()[dc * 128:(dc + 1) * 128,
                                            th * TH:(th + 1) * TH])
                    nc.vector.tensor_add(
                        vT[:, dc, th * TH:(th + 1) * TH],
                        vT[:, dc, th * TH:(th + 1) * TH].bitcast(F32),
                        pch[:])
                for dc in range(ND):
                    tp = pmm([128, 8, 128], F32R)
                    for k in range(8):
                        n = th * 8 + k
                        nc.tensor.transpose(out=tp[:, k, :],
                                            in_=vT[:, dc, n * 128:(n + 1) * 128],
                                            identity=ident_r[:])
                    nc.vector.tensor_copy(
                        vp_td[:, th * 8:(th + 1) * 8,
                              dc * 128:(dc + 1) * 128], tp[:])

            # ================================ layers ================================
            for layer in range(N_LAYERS):
                # ---- A (layer 0 only; later layers fold A into E bodies) ----
                if layer == 0:
                    for th in range(2):
                        phaseA(th)

                # ---- B: x = relu(v @ Dx) (kT layout); RoPE -> q; spill x ----
                for i in range(4):
                    cos_t = w8()
                    nc.sync.dma_start(cos_t[:], cos_d.ap()[i])
                    sin_t = w8()
                    nc.sync.dma_start(sin_t[:], sin_d.ap()[i])
                    xts = {}
                    for ii in (i, i + 4):
                        xt = w8()
                        xts[ii] = xt
                        for th in range(2):
                            px = pmm()
                            for dc in range(ND):
                                for ns in range(NS):
                                    nc.tensor.matmul(
                                        px[:, ns * 512:(ns + 1) * 512],
                                        dx_sb[:, dc, ii * 128:(ii + 1) * 128],
                                        vT[:, dc, th * TH + ns * 512:
                                           th * TH + (ns + 1) * 512],
                                        start=(dc == 0), stop=(dc == ND - 1))
                            nc.scalar.activation(xt[:, th * TH:(th + 1) * TH],
                                                 px[:], AF.Relu)
                        nc.sync.dma_start(xspill[ii], xt[:])
                    xi, xj = xts[i], xts[i + 4]
                    m1 = w8()
                    nc.vector.tensor_mul(m1[:], xi[:], cos_t[:])
                    m2 = w8()
                    nc.vector.tensor_mul(m2[:], xj[:], sin_t[:])
                    nc.vector.tensor_sub(qT[:, i], m1[:], m2[:])
                    m3 = w8()
                    nc.vector.tensor_mul(m3[:], xj[:], cos_t[:])
                    m4 = w8()
                    nc.vector.tensor_mul(m4[:], xi[:], sin_t[:])
                    nc.vector.tensor_add(qT[:, i + 4], m3[:], m4[:])

                # ---- C+D per t-half ----
                for th in range(2):
                    t0 = th * TH
                    # C: scores[s_j, t-half] -> aT += vp_td[s_j]^T @ scores
                    pa = psacc.tile([128, ND, TH], F32, tag="acc")
                    for j in range(NT):
                        pscr = pmm()
                        for kc in range(NKT):
                            for ns in range(NS):
                                nc.tensor.matmul(
                                    pscr[:, ns * 512:(ns + 1) * 512],
                                    qT[:, kc, j * 128:(j + 1) * 128],
                                    qT[:, kc, t0 + ns * 512:t0 + (ns + 1) * 512],
                                    start=(kc == 0), stop=(kc == NKT - 1))
                        scr = t4(F32R)
                        nc.vector.tensor_copy(scr[:], pscr[:])
                        for dc in range(ND):
                            for ns in range(NS):
                                nc.tensor.matmul(
                                    pa[:, dc, ns * 512:(ns + 1) * 512],
                                    vp_td[:, j, dc * 128:(dc + 1) * 128],
                                    scr[:, ns * 512:(ns + 1) * 512],
                                    start=(j == 0), stop=(j == NT - 1),
                                    skip_group_check=True)
                    aT = w8n(F32R)
                    for dc in range(ND):
                        nc.vector.tensor_copy(aT[:, dc], pa[:, dc])
                    # ln(a) over d (partition dim)
                    s1, s2 = colsums(aT, 0)
                    negmean, rstd = pstats(s1, s2, D)
                    nm_b = bcast(negmean)
                    rs_b = bcast(rstd)
                    lnA = w8n(F32R)
                    for dc in range(ND):
                        cent = t4()
                        nc.vector.tensor_add(cent[:], aT[:, dc].bitcast(F32), nm_b[:])
                        nc.vector.tensor_mul(lnA[:, dc], cent[:], rs_b[:])

                    # D: y_i = relu(lnA @ Dy_i) * x_i ; yET += E_i^T-style accum
                    pye = psacc.tile([128, ND, TH], F32, tag="acc")
                    for i in range(NKT):
                        py = pmm()
                        for dc in range(ND):
                            for ns in range(NS):
                                nc.tensor.matmul(
                                    py[:, ns * 512:(ns + 1) * 512],
                                    dy_sb[:, dc, i * 128:(i + 1) * 128],
                                    lnA[:, dc, ns * 512:(ns + 1) * 512],
                                    start=(dc == 0), stop=(dc == ND - 1))
                        xr = t4()
                        nc.sync.dma_start(xr[:], xspill[i, :, t0:t0 + TH])
                        yt = t4(F32R)
                        nc.vector.scalar_tensor_tensor(
                            out=yt[:], in0=py[:], scalar=0.0, in1=xr[:],
                            op0=OP.max, op1=OP.mult)
                        for dc in range(ND):
                            for ns in range(NS):
                                nc.tensor.matmul(
                                    pye[:, dc, ns * 512:(ns + 1) * 512],
                                    e_sb[:, i, dc * 128:(dc + 1) * 128],
                                    yt[:, ns * 512:(ns + 1) * 512],
                                    start=(i == 0), stop=(i == NKT - 1),
                                    skip_group_check=True)
                    ye = w8n(F32)
                    for dc in range(ND):
                        nc.vector.tensor_copy(ye[:, dc], pye[:, dc])
                        nc.sync.dma_start(cc_in[th][:][dc], ye[:, dc])
                    nc.gpsimd.collective_compute(
                        "AllReduce", OP.add, replica_groups=groups,
                        ins=[cc_in[th][:].opt()], outs=[cc_out[th][:].opt()])

                # ---- E: u = ln(sum); w = vp + u; v = ln(w) (dT layout) ----
                for th in range(2):
                    t0 = th * TH
                    uT = w8n(F32R)
                    for dc in range(ND):
                        nc.sync.dma_start(uT[:, dc].bitcast(F32), cc_out[th][:][dc])
                    s1, s2 = colsums(uT, 0)
                    negmean, rstd = pstats(s1, s2, D)
                    nm_b = bcast(negmean)
                    rs_b = bcast(rstd)
                    for dc in range(ND):
                        cent = t4()
                        nc.vector.tensor_add(cent[:], uT[:, dc].bitcast(F32), nm_b[:])
                        lnu = t4()
                        nc.vector.tensor_mul(lnu[:], cent[:], rs_b[:])
                        nc.vector.tensor_add(vT[:, dc, t0:t0 + TH],
                                             vT[:, dc, t0:t0 + TH].bitcast(F32),
                                             lnu[:])
                    s1, s2 = colsums(vT, t0)
                    negmean, rstd = pstats(s1, s2, D)
                    nm_b = bcast(negmean)
                    rs_b = bcast(rstd)
                    for dc in range(ND):
                        cent = t4()
                        nc.vector.tensor_add(cent[:],
                                             vT[:, dc, t0:t0 + TH].bitcast(F32),
                                             nm_b[:])
                        nc.vector.tensor_mul(vT[:, dc, t0:t0 + TH],
                                             cent[:], rs_b[:])
                    if layer < N_LAYERS - 1:
                        phaseA(th)

            # debug dump of final vT
            for dc in range(ND):
                nc.sync.dma_start(vdbg_d.ap()[dc], vT[:, dc].bitcast(F32))

            # ============= readout: logitsT = (v @ readout)^T, V-sharded =============
            if DO_READOUT:
                nvb = (VS + 127) // 128
                for vb in range(nvb):
                    m = min(128, VS - vb * 128)
                    ro_sb = t4p.tile([128, ND, 128], F32R, tag="ro",
                                     bufs=2, name=_nm("ro_"))
                    for dc in range(ND):
                        nc.sync.dma_start(
                            ro_sb[:, dc, :m],
                            ro_d.ap()[dc * 128:(dc + 1) * 128,
                                      vb * 128:vb * 128 + m])
                    lo = w8()
                    for th in range(2):
                        pl = pmm()
                        for dc in range(ND):
                            for ns in range(NS):
                                nc.tensor.matmul(
                                    pl[:m, ns * 512:(ns + 1) * 512],
                                    ro_sb[:, dc, :m],
                                    vT[:, dc, th * TH + ns * 512:
                                       th * TH + (ns + 1) * 512],
                                    start=(dc == 0), stop=(dc == ND - 1))
                        nc.scalar.copy(lo[:m, th * TH:(th + 1) * TH], pl[:m])
                    nc.sync.dma_start(out_d.ap()[vb * 128:vb * 128 + m, :], lo[:m])

    nc.compile()
    return nc


_NC_CACHE = None


def _get_nc():
    global _NC_CACHE
    if _NC_CACHE is None:
        nc = bacc.Bacc("TRN2", target_bir_lowering=False, debug=False, num_devices=8)
        _NC_CACHE = build(nc)
    return _NC_CACHE


def _rope_tables():
    # match the jax reference: float32 angle computation
    inv_freq = (1.0 / (10000.0 ** (np.arange(0, K, 2, dtype=np.float32)
                                   / np.float32(K)))).astype(np.float32)
    t = np.arange(T, dtype=np.float32)
    freqs = (t[:, None] * inv_freq[None, :]).astype(np.float32)  # [T, K/2]
    cos = np.cos(freqs).astype(np.float32)
    sin = np.sin(freqs).astype(np.float32)
    cosT = np.ascontiguousarray(cos.T).reshape(4, 128, T)
    sinT = np.ascontiguousarray(sin.T).reshape(4, 128, T)
    return cosT, sinT


def kernel(input_, emb, pos, Dx, Dy, E, readout):
    input_ = np.asarray(input_)
    emb = np.ascontiguousarray(np.asarray(emb, dtype=np.float32))
    pos = np.asarray(pos, dtype=np.float32)
    Dx = np.asarray(Dx, dtype=np.float32)
    Dy = np.asarray(Dy, dtype=np.float32)
    E = np.asarray(E, dtype=np.float32)
    readout = np.asarray(readout, dtype=np.float32)

    nc = _get_nc()
    cosT, sinT = _rope_tables()
    posT = np.ascontiguousarray(pos.T)

    in_maps = []
    for c in range(8):
        b, h = divmod(c, 4)
        in_maps.append({
            "tok": np.ascontiguousarray(input_[b].astype(np.int32)),
            "emb": emb,
            "posT": posT,
            "dx": np.ascontiguousarray(Dx[h]),
            "dy": np.ascontiguousarray(Dy[h]),
            "eh": np.ascontiguousarray(E[h * K:(h + 1) * K]),
            "ro": np.ascontiguousarray(readout[:, h * VS:(h + 1) * VS]),
            "cosh": cosT,
            "sinh": sinT,
        })
    trace = os.environ.get("KRN_TRACE", "0") == "1"
    res = run_bass_kernel_spmd(nc, in_maps, list(range(8)), trace=trace)
    out = np.empty((B, T, V), dtype=np.float32)
    for c in range(8):
        b, h = divmod(c, 4)
        out[b, :, h * VS:(h + 1) * VS] = res.results[c]["logitsT"].T
    kernel._last_results = res
    return out



# revision 12
# speedup vs baseline: 1.4852x; 1.4852x over previous
"""Trainium2 Bass kernel for nn_BDH_4406636445711 (dense transformer).

Sharding: 8 cores = data-parallel over B(2) x tensor-parallel over H(4).
Core c handles (b = c//4, h = c%4): its head's Dx/Dy slices, E rows, and a
V/4 shard of the readout. Per layer the y@E partial is AllReduced within
each b-group of 4 cores; v stays replicated inside the group. The host
stitches the 8 per-core [VS, T] logit shards into the full [B, T, V].

Key algorithmic change vs the naive reference: there is no softmax, so
  a = (q @ q^T) @ v  ==  q @ (q^T @ v)
which replaces the [T,T] scores matmul (8.6 GF/core/layer) with two
[T,K]x[K,D]-sized matmuls (1.1 GF each). M = q^T(v+pos) is computed from
td-layout tiles built by DMA-transpose (bf16 xbar path), costing zero PE
time. q/x/Dx/Dy/E/M run in bf16 (matmul rate is identical to f32r, DVE
gets 4x, SBUF pressure halves); the residual stream v and all LayerNorm
statistics stay f32/f32r. The per-token LN(a) rstd is folded into the
y@E psum->sbuf scale; the mean is centered with one bf16 DVE pass.
"""

import os
import sys

sys.path.insert(0, "/opt/trn_rl_repo")

import ml_dtypes
import numpy as np

import concourse.bass as bass
import concourse.tile as tile
from concourse import bacc, mybir
from concourse.bass_utils import run_bass_kernel_spmd
from concourse.masks import make_identity
from concourse import library_config

F32 = mybir.dt.float32
F32R = mybir.dt.float32r
BF16 = mybir.dt.bfloat16
I32 = mybir.dt.int32
AF = mybir.ActivationFunctionType
OP = mybir.AluOpType

B, T, H, D, K, V, L = 2, 2048, 4, 256, 1024, 32000, 6
VS = V // 4          # vocab shard per core within a b-group
EPS = 1e-5
NT = T // 128        # 16 token tiles
NKT = K // 128       # 8 k' tiles
ND = D // 128        # 2 d tiles
TH = T // 2          # t-half = 1024
NS = TH // 512       # 512-wide matmul chunks per t-half

N_LAYERS = int(os.environ.get("KRN_LAYERS", str(L)))
DO_READOUT = os.environ.get("KRN_READOUT", "1") == "1"


def build(nc):
    # ---- DRAM parameters (per core) ----
    tok_d = nc.dram_tensor("tok", [T], I32, kind="ExternalInput")
    emb_d = nc.dram_tensor("emb", [V, D], F32, kind="ExternalInput")
    posT_d = nc.dram_tensor("posT", [D, T], F32, kind="ExternalInput")
    dx_d = nc.dram_tensor("dx", [D, K], BF16, kind="ExternalInput")
    dy_d = nc.dram_tensor("dy", [D, K], BF16, kind="ExternalInput")
    e_d = nc.dram_tensor("eh", [K, D], BF16, kind="ExternalInput")
    ro_d = nc.dram_tensor("ro", [D, VS], F32R, kind="ExternalInput")
    cos_d = nc.dram_tensor("cosh", [4, 128, T], BF16, kind="ExternalInput")
    sin_d = nc.dram_tensor("sinh", [4, 128, T], BF16, kind="ExternalInput")
    out_d = nc.dram_tensor("logitsT", [VS, T], F32, kind="ExternalOutput")

    groups = [[0, 1, 2, 3], [4, 5, 6, 7]]

    with tile.TileContext(nc) as tc:
        with (
            nc.allow_low_precision(reason="bf16/f32r rounding is intentional"),
            tc.tile_pool(name="persist", bufs=1) as pp,
            tc.tile_pool(name="work", bufs=1) as wp,
            tc.tile_pool(name="psmm", bufs=2, space="PSUM") as psmm,
            tc.tile_pool(name="psacc", bufs=1, space="PSUM") as psacc,
            tc.tile_pool(name="dram", bufs=1, space="DRAM") as dpool,
        ):
            _ctr = [0]

            def _nm(p):
                _ctr[0] += 1
                return f"{p}{_ctr[0]}"

            # pool helpers -- tags control slot sharing
            def xt_t():
                # [128, T] bf16 (x tiles, emb-phase scratch)
                return wp.tile([128, T], BF16, tag="xt", bufs=3, name=_nm("xt_"))

            def rope_t():
                return wp.tile([128, T], BF16, tag="rope", bufs=2, name=_nm("rp_"))

            def trig_t():
                return wp.tile([128, T], BF16, tag="trig", bufs=3, name=_nm("tg_"))

            def qtd_t():
                return wp.tile([128, 8, 128], BF16, tag="qtd", bufs=3,
                               name=_nm("qtd_"))

            def f4_t():
                # [128, TH] f32 scratch (pos chunks, E-phase temps)
                return wp.tile([128, TH], F32, tag="f4", bufs=2, name=_nm("f4_"))

            def xr_t():
                # [128, TH] bf16 (x reload, relu result, gated y)
                return wp.tile([128, TH], BF16, tag="xr", bufs=5, name=_nm("xr_"))

            def aT_t():
                return wp.tile([128, ND, TH], BF16, tag="aT", bufs=2,
                               name=_nm("aT_"))

            def sq_t():
                return wp.tile([128, ND, TH], BF16, tag="sq", bufs=2,
                               name=_nm("sq_"))

            def rs_t():
                # [128, TH] f32 broadcast tiles (rstd)
                return wp.tile([128, TH], F32, tag="rs", bufs=2, name=_nm("rs_"))

            def nmb_t():
                # [128, TH] bf16 broadcast tiles (negmean)
                return wp.tile([128, TH], BF16, tag="nmb", bufs=2, name=_nm("nm_"))

            def w8_t():
                # [128, ND, TH] f32 (uT, ye, readout logits)
                return wp.tile([128, ND, TH], F32, tag="w8", bufs=3,
                               name=_nm("w8_"))

            def st_t(dt=F32):
                return wp.tile([1, TH], dt, tag="st", bufs=3, name=_nm("st_"))

            def pmm(shape=None, dt=F32):
                return psmm.tile(shape or [128, TH], dt, tag="mm", name=_nm("mm_"))

            # ---- constants ----
            ident_f = wp.tile([128, 128], F32, tag="idf", bufs=1)
            make_identity(nc, ident_f[:])
            ident_r = pp.tile([128, 128], F32R)
            nc.vector.tensor_copy(ident_r[:], ident_f[:])
            ones_pf = pp.tile([128, 1], F32)
            nc.vector.memset(ones_pf[:], 1.0)
            ones_p = pp.tile([128, 1], F32R)
            nc.vector.tensor_copy(ones_p[:], ones_pf[:])
            ones_pb = pp.tile([128, 1], BF16)
            nc.vector.tensor_copy(ones_pb[:], ones_pf[:])
            ones_cf = pp.tile([1, 128], F32)
            nc.vector.memset(ones_cf[:], 1.0)
            ones_c = pp.tile([1, 128], F32R)
            nc.vector.tensor_copy(ones_c[:], ones_cf[:])
            eps_p = pp.tile([128, 1], F32)
            nc.vector.memset(eps_p[:], EPS)
            eps_1 = pp.tile([1, 1], F32)
            nc.vector.memset(eps_1[:], EPS)
            nc.gpsimd.load_library(library_config.attn)

            # ---- persistent tensors ----
            vT = pp.tile([128, ND, T], F32R)       # v (dT layout), f32 bits
            vpb = pp.tile([128, ND, T], BF16)      # bf16 copy of v (dT layout)
            qT = pp.tile([128, NKT, T], BF16)      # q (kT layout)
            vp_td = pp.tile([128, NT, ND, 128], BF16)  # v+pos (td, xbar tiling)
            m_sb = pp.tile([128, NKT, D], BF16)    # M = q^T (v+pos)  [k, d]
            dx_sb = pp.tile([128, ND, K], BF16)
            nc.sync.dma_start(dx_sb[:], dx_d.ap().rearrange("(c p) k -> p c k", p=128))
            dy_sb = pp.tile([128, ND, K], BF16)
            nc.sync.dma_start(dy_sb[:], dy_d.ap().rearrange("(c p) k -> p c k", p=128))
            e_sb = pp.tile([128, NKT, D], BF16)
            nc.sync.dma_start(e_sb[:], e_d.ap().rearrange("(c p) d -> p c d", p=128))

            # ---- internal DRAM ----
            xspill = dpool.tile([NKT, 128, T], BF16, tag="xspill")
            cc_in = [dpool.tile([ND, 128, TH], F32, tag=f"cci{i}", name=f"cci{i}")
                     for i in range(2)]
            cc_out = [dpool.tile([ND, 128, TH], F32, tag=f"cco{i}", name=f"cco{i}")
                      for i in range(2)]

            def pstats(negmean_src_ps, s2_src_ps, n):
                """negmean=-s1/n, rstd=1/sqrt(s2/n-mean^2+eps) as [1, TH] f32r."""
                negmean = st_t(F32R)
                nc.vector.tensor_scalar_mul(negmean[:], negmean_src_ps, -1.0 / n)
                m2 = st_t()
                nc.vector.tensor_mul(m2[:], negmean[:].bitcast(F32),
                                     negmean[:].bitcast(F32))
                var = st_t()
                nc.vector.scalar_tensor_tensor(
                    out=var[:], in0=s2_src_ps, scalar=1.0 / n, in1=m2[:],
                    op0=OP.mult, op1=OP.subtract)
                lnv = st_t()
                nc.scalar.activation(lnv[:], var[:], AF.Ln, bias=eps_1[:])
                rstd = st_t(F32R)
                nc.scalar.activation(rstd[:], lnv[:], AF.Exp, scale=-0.5)
                return negmean, rstd

            def colsum(mov_fn, ones):
                """s[t] = sum_d mov[d, t] over ND tiles -> [1, TH] psum AP.

                mov_fn(dc, lo, hi) must return a [128, hi-lo] SBUF AP."""
                s = pmm([1, TH])
                for dc in range(ND):
                    for ns in range(NS):
                        nc.tensor.matmul(
                            s[:1, ns * 512:(ns + 1) * 512], ones[:],
                            mov_fn(dc, ns * 512, (ns + 1) * 512),
                            start=(dc == 0), stop=(dc == ND - 1),
                            skip_group_check=True)
                return s[:1, :]

            def bcast(vec):
                """PE rank-1 broadcast of a [1, TH] f32r vector to [128, TH] psum."""
                out = pmm()
                for ns in range(NS):
                    nc.tensor.matmul(out[:, ns * 512:(ns + 1) * 512], ones_c[:],
                                     vec[:, ns * 512:(ns + 1) * 512],
                                     start=True, stop=True)
                return out

            # ============ embedding gather + LN -> v0 -> transpose to vT ============
            idx = pp.tile([128, NT], I32)
            nc.sync.dma_start(idx[:], tok_d.ap().rearrange("(n p) -> p n", p=128))
            for n in range(NT):
                gat = wp.tile([128, D], F32, tag="xt", bufs=3, name=_nm("g_"))
                nc.gpsimd.indirect_dma_start(
                    out=gat[:], out_offset=None, in_=emb_d.ap(),
                    in_offset=bass.IndirectOffsetOnAxis(ap=idx[:, n:n + 1], axis=0),
                )
                stats = wp.tile([128, 6], F32, tag="bst", bufs=2, name=_nm("g_"))
                nc.vector.bn_stats(out=stats[:], in_=gat[:])
                mv = wp.tile([128, 2], F32, tag="bmv", bufs=2, name=_nm("g_"))
                nc.vector.bn_aggr(out=mv[:], in_=stats[:])
                std = wp.tile([128, 1], F32, tag="bsd", bufs=2, name=_nm("g_"))
                nc.scalar.activation(std[:], mv[:, 1:2], AF.Sqrt, bias=eps_p[:])
                rstd = wp.tile([128, 1], F32, tag="brs", bufs=2, name=_nm("g_"))
                nc.vector.reciprocal(rstd[:], std[:])
                v0 = wp.tile([128, D], F32R, tag="xt", bufs=3, name=_nm("g_"))
                nc.vector.tensor_scalar(
                    out=v0[:], in0=gat[:], scalar1=mv[:, 0:1], scalar2=rstd[:],
                    op0=OP.subtract, op1=OP.mult)
                for dc in range(ND):
                    tp = pmm([128, 128], F32R)
                    nc.tensor.transpose(out=tp[:], in_=v0[:, dc * 128:(dc + 1) * 128],
                                        identity=ident_r[:])
                    nc.vector.tensor_copy(vT[:, dc, n * 128:(n + 1) * 128], tp[:])

            def phaseA(th):
                """v[:, th-half] += pos; refresh vpb + vp_td for that half."""
                t0 = th * TH
                for dc in range(ND):
                    pch = f4_t()
                    nc.sync.dma_start(
                        pch[:], posT_d.ap()[dc * 128:(dc + 1) * 128, t0:t0 + TH])
                    nc.vector.tensor_add(
                        vT[:, dc, t0:t0 + TH],
                        vT[:, dc, t0:t0 + TH].bitcast(F32), pch[:])
                refresh_half(th)

            def refresh_half(th):
                """vpb = bf16(vT) and vp_td = xbar-transpose(vpb) for a t-half."""
                t0 = th * TH
                for dc in range(ND):
                    nc.scalar.copy(vpb[:, dc, t0:t0 + TH],
                                   vT[:, dc, t0:t0 + TH].bitcast(F32))
                    nc.sync.dma_start(
                        out=vp_td[:, th * 8:(th + 1) * 8, dc, :],
                        in_=vpb[:, dc, t0:t0 + TH],
                        transpose=True)

            # ================================ layers ================================
            for layer in range(N_LAYERS):
                if layer == 0:
                    for th in range(2):
                        phaseA(th)

                # ---- B: x = relu((v+pos) @ Dx) kT layout; RoPE -> q; spill x ----
                for i in range(4):
                    cos_t = trig_t()
                    nc.sync.dma_start(cos_t[:], cos_d.ap()[i])
                    sin_t = trig_t()
                    nc.sync.dma_start(sin_t[:], sin_d.ap()[i])
                    xts = {}
                    for ii in (i, i + 4):
                        xt = xt_t()
                        xts[ii] = xt
                        for th in range(2):
                            px = pmm()
                            for dc in range(ND):
                                for ns in range(NS):
                                    nc.tensor.matmul(
                                        px[:, ns * 512:(ns + 1) * 512],
                                        dx_sb[:, dc, ii * 128:(ii + 1) * 128],
                                        vpb[:, dc, th * TH + ns * 512:
                                            th * TH + (ns + 1) * 512],
                                        start=(dc == 0), stop=(dc == ND - 1))
                            nc.scalar.activation(xt[:, th * TH:(th + 1) * TH],
                                                 px[:], AF.Relu)
                        nc.sync.dma_start(xspill[ii], xt[:])
                    xi, xj = xts[i], xts[i + 4]
                    m1 = rope_t()
                    nc.vector.tensor_mul(m1[:], xi[:], cos_t[:])
                    m2 = rope_t()
                    nc.vector.tensor_mul(m2[:], xj[:], sin_t[:])
                    nc.vector.tensor_sub(qT[:, i], m1[:], m2[:])
                    m3 = rope_t()
                    nc.vector.tensor_mul(m3[:], xj[:], cos_t[:])
                    m4 = rope_t()
                    nc.vector.tensor_mul(m4[:], xi[:], sin_t[:])
                    nc.vector.tensor_add(qT[:, i + 4], m3[:], m4[:])

                # ---- C1: M[k,d] = sum_t q[t,k] (v+pos)[t,d] via xbar q_td ----
                pm = psacc.tile([128, NKT, D], F32, tag="acc", name=_nm("pm_"))
                for kc in (0, 4, 1, 5, 2, 6, 3, 7):
                    for th in range(2):
                        qtd = qtd_t()
                        nc.sync.dma_start(
                            out=qtd[:],
                            in_=qT[:, kc, th * TH:(th + 1) * TH],
                            transpose=True)
                        for n in range(8):
                            nc.tensor.matmul(
                                pm[:, kc, :], qtd[:, n, :],
                                vp_td[:, th * 8 + n, :, :],
                                start=(th == 0 and n == 0),
                                stop=(th == 1 and n == 7),
                                skip_group_check=True)
                    nc.scalar.copy(m_sb[:, kc, :], pm[:, kc, :])

                # ---- C2 + LN(a) + D per t-half ----
                for th in range(2):
                    t0 = th * TH
                    # C2: aT[d, t-half] = sum_k M[k,d] q[k,t]
                    aT = aT_t()
                    sq = sq_t()
                    for dc in range(ND):
                        pa = pmm()
                        for kc in range(NKT):
                            for ns in range(NS):
                                nc.tensor.matmul(
                                    pa[:, ns * 512:(ns + 1) * 512],
                                    m_sb[:, kc, dc * 128:(dc + 1) * 128],
                                    qT[:, kc, t0 + ns * 512:t0 + (ns + 1) * 512],
                                    start=(kc == 0), stop=(kc == NKT - 1))
                        nc.scalar.copy(aT[:, dc], pa[:])
                        nc.scalar.activation(sq[:, dc], pa[:], AF.Square)
                    # LN(a) stats; rstd folded into the y@E output scale
                    s1 = colsum(lambda dc, lo, hi: aT[:, dc, lo:hi], ones_pb)
                    s2 = colsum(lambda dc, lo, hi: sq[:, dc, lo:hi], ones_pb)
                    negmean, rstd = pstats(s1, s2, D)
                    nm_ps = bcast(negmean)
                    nm_s = nmb_t()
                    nc.scalar.copy(nm_s[:], nm_ps[:])
                    rs_ps = bcast(rstd)
                    rs_s = rs_t()
                    nc.scalar.copy(rs_s[:], rs_ps[:])
                    for dc in range(ND):
                        nc.vector.tensor_add(aT[:, dc], aT[:, dc], nm_s[:])

                    # D: y_i = relu((a-mu) @ Dy_i) * x_i ; pye = sum_i E_i^T y_i
                    pye = psacc.tile([128, ND, TH], F32, tag="acc",
                                     name=_nm("pye_"))
                    for i in range(NKT):
                        py = pmm()
                        for dc in range(ND):
                            for ns in range(NS):
                                nc.tensor.matmul(
                                    py[:, ns * 512:(ns + 1) * 512],
                                    dy_sb[:, dc, i * 128:(i + 1) * 128],
                                    aT[:, dc, ns * 512:(ns + 1) * 512],
                                    start=(dc == 0), stop=(dc == ND - 1))
                        rl = xr_t()
                        nc.scalar.activation(rl[:], py[:], AF.Relu)
                        xr = xr_t()
                        nc.sync.dma_start(xr[:], xspill[i, :, t0:t0 + TH])
                        yt = xr_t()
                        nc.vector.tensor_mul(yt[:], rl[:], xr[:])
                        for dc in range(ND):
                            for ns in range(NS):
                                nc.tensor.matmul(
                                    pye[:, dc, ns * 512:(ns + 1) * 512],
                                    e_sb[:, i, dc * 128:(dc + 1) * 128],
                                    yt[:, ns * 512:(ns + 1) * 512],
                                    start=(i == 0), stop=(i == NKT - 1),
                                    skip_group_check=True)
                    ye = w8_t()
                    for dc in range(ND):
                        nc.vector.tensor_mul(ye[:, dc], pye[:, dc], rs_s[:])
                        nc.sync.dma_start(cc_in[th][:][dc], ye[:, dc])
                    nc.gpsimd.collective_compute(
                        "AllReduce", OP.add, replica_groups=groups,
                        ins=[cc_in[th][:].opt()], outs=[cc_out[th][:].opt()])

                # ---- E: u = ln(sum); w = vp + u; v = ln(w) (dT layout) ----
                for th in range(2):
                    t0 = th * TH
                    uT = w8_t()
                    squ = sq_t()
                    for dc in range(ND):
                        nc.sync.dma_start(uT[:, dc], cc_out[th][:][dc])
                        nc.scalar.activation(squ[:, dc], uT[:, dc], AF.Square)
                    s1 = colsum(lambda dc, lo, hi: uT[:, dc, lo:hi].bitcast(F32R),
                                ones_p)
                    s2 = colsum(lambda dc, lo, hi: squ[:, dc, lo:hi], ones_pb)
                    negmean, rstd = pstats(s1, s2, D)
                    nm_u = bcast(negmean)
                    rs_u = bcast(rstd)
                    for dc in range(ND):
                        t1 = f4_t()
                        nc.vector.tensor_add(t1[:], uT[:, dc], nm_u[:])
                        nc.vector.tensor_mul(t1[:], t1[:], rs_u[:])
                        nc.vector.tensor_add(vT[:, dc, t0:t0 + TH],
                                             vT[:, dc, t0:t0 + TH].bitcast(F32),
                                             t1[:])
                    sqw = sq_t()
                    for dc in range(ND):
                        nc.scalar.activation(sqw[:, dc],
                                             vT[:, dc, t0:t0 + TH].bitcast(F32),
                                             AF.Square)
                    s1 = colsum(lambda dc, lo, hi: vT[:, dc, t0 + lo:t0 + hi],
                                ones_p)
                    s2 = colsum(lambda dc, lo, hi: sqw[:, dc, lo:hi], ones_pb)
                    negmean, rstd = pstats(s1, s2, D)
                    nm_w = bcast(negmean)
                    rs_w = bcast(rstd)
                    for dc in range(ND):
                        nc.vector.tensor_add(vT[:, dc, t0:t0 + TH],
                                             vT[:, dc, t0:t0 + TH].bitcast(F32),
                                             nm_w[:])
                        nc.vector.tensor_mul(vT[:, dc, t0:t0 + TH],
                                             vT[:, dc, t0:t0 + TH].bitcast(F32),
                                             rs_w[:])
                    if layer < N_LAYERS - 1:
                        phaseA(th)

            # ============= readout: logitsT = (v @ readout)^T, V-sharded =============
            if DO_READOUT:
                nvb = (VS + 127) // 128
                for vb in range(nvb):
                    m = min(128, VS - vb * 128)
                    ro_sb = wp.tile([128, ND, 128], F32R, tag="ro",
                                    bufs=2, name=_nm("ro_"))
                    for dc in range(ND):
                        nc.sync.dma_start(
                            ro_sb[:, dc, :m],
                            ro_d.ap()[dc * 128:(dc + 1) * 128,
                                      vb * 128:vb * 128 + m])
                    lo = w8_t()
                    for th in range(2):
                        pl = pmm()
                        for dc in range(ND):
                            for ns in range(NS):
                                nc.tensor.matmul(
                                    pl[:m, ns * 512:(ns + 1) * 512],
                                    ro_sb[:, dc, :m],
                                    vT[:, dc, th * TH + ns * 512:
                                       th * TH + (ns + 1) * 512],
                                    start=(dc == 0), stop=(dc == ND - 1))
                        nc.scalar.copy(lo[:m, th], pl[:m])
                    nc.sync.dma_start(
                        out_d.ap()[vb * 128:vb * 128 + m, :], lo[:m])

    nc.compile()
    return nc


_NC_CACHE = None


def _get_nc():
    global _NC_CACHE
    if _NC_CACHE is None:
        nc = bacc.Bacc("TRN2", target_bir_lowering=False, debug=False, num_devices=8)
        _NC_CACHE = build(nc)
    return _NC_CACHE


def _rope_tables():
    # match the jax reference: float32 angle computation
    inv_freq = (1.0 / (10000.0 ** (np.arange(0, K, 2, dtype=np.float32)
                                   / np.float32(K)))).astype(np.float32)
    t = np.arange(T, dtype=np.float32)
    freqs = (t[:, None] * inv_freq[None, :]).astype(np.float32)  # [T, K/2]
    cos = np.cos(freqs).astype(ml_dtypes.bfloat16)
    sin = np.sin(freqs).astype(ml_dtypes.bfloat16)
    cosT = np.ascontiguousarray(cos.T).reshape(4, 128, T)
    sinT = np.ascontiguousarray(sin.T).reshape(4, 128, T)
    return cosT, sinT


def kernel(input_, emb, pos, Dx, Dy, E, readout):
    input_ = np.asarray(input_)
    emb = np.ascontiguousarray(np.asarray(emb, dtype=np.float32))
    pos = np.asarray(pos, dtype=np.float32)
    Dx = np.asarray(Dx, dtype=np.float32)
    Dy = np.asarray(Dy, dtype=np.float32)
    E = np.asarray(E, dtype=np.float32)
    readout = np.asarray(readout, dtype=np.float32)

    nc = _get_nc()
    cosT, sinT = _rope_tables()
    posT = np.ascontiguousarray(pos.T)

    in_maps = []
    for c in range(8):
        b, h = divmod(c, 4)
        in_maps.append({
            "tok": np.ascontiguousarray(input_[b].astype(np.int32)),
            "emb": emb,
            "posT": posT,
            "dx": np.ascontiguousarray(Dx[h]).astype(ml_dtypes.bfloat16),
            "dy": np.ascontiguousarray(Dy[h]).astype(ml_dtypes.bfloat16),
            "eh": np.ascontiguousarray(E[h * K:(h + 1) * K]).astype(
                ml_dtypes.bfloat16),
            "ro": np.ascontiguousarray(readout[:, h * VS:(h + 1) * VS]),
            "cosh": cosT,
            "sinh": sinT,
        })
    trace = os.environ.get("KRN_TRACE", "0") == "1"
    res = run_bass_kernel_spmd(nc, in_maps, list(range(8)), trace=trace)
    out = np.empty((B, T, V), dtype=np.float32)
    for c in range(8):
        b, h = divmod(c, 4)
        out[b, :, h * VS:(h + 1) * VS] = res.results[c]["logitsT"].T
    kernel._last_results = res
    return out


# revision 13
# speedup vs baseline: 1.5187x; 1.0225x over previous
"""Trainium2 Bass kernel for nn_BDH_4406636445711 (dense transformer).

Sharding: 8 cores = data-parallel over B(2) x tensor-parallel over H(4).
Core c handles (b = c//4, h = c%4): its head's Dx/Dy slices, E rows, and a
V/4 shard of the readout. Per layer the y@E partial is AllReduced within
each b-group of 4 cores; v stays replicated inside the group. The host
stitches the 8 per-core [VS, T] logit shards into the full [B, T, V].

Key algorithmic change vs the naive reference: there is no softmax, so
  a = (q @ q^T) @ v  ==  q @ (q^T @ v)
which replaces the [T,T] scores matmul (8.6 GF/core/layer) with two
[T,K]x[K,D]-sized matmuls (1.1 GF each). M = q^T(v+pos) is computed from
td-layout tiles built by DMA-transpose (bf16 xbar path), costing zero PE
time. q/x/Dx/Dy/E/M and the readout run in bf16 (matmul rate matches
f32r, DVE gets 4x, SBUF pressure halves); the residual stream v and all
LayerNorm statistics stay f32/f32r. The per-token LN(a) rstd is folded
into the y@E psum->sbuf scale; the mean is centered with one bf16 DVE
pass. The B phase (x/q production) is split per t-half and runs between
the two E phases so the second AllReduce of each layer is hidden under
next-layer compute.
"""

import os
import sys

sys.path.insert(0, "/opt/trn_rl_repo")

import ml_dtypes
import numpy as np

import concourse.bass as bass
import concourse.tile as tile
from concourse import bacc, mybir
from concourse.bass_utils import run_bass_kernel_spmd
from concourse.masks import make_identity
from concourse import library_config

F32 = mybir.dt.float32
F32R = mybir.dt.float32r
BF16 = mybir.dt.bfloat16
I32 = mybir.dt.int32
AF = mybir.ActivationFunctionType
OP = mybir.AluOpType

B, T, H, D, K, V, L = 2, 2048, 4, 256, 1024, 32000, 6
VS = V // 4          # vocab shard per core within a b-group
EPS = 1e-5
NT = T // 128        # 16 token tiles
NKT = K // 128       # 8 k' tiles
ND = D // 128        # 2 d tiles
TH = T // 2          # t-half = 1024
NS = TH // 512       # 512-wide matmul chunks per t-half

N_LAYERS = int(os.environ.get("KRN_LAYERS", str(L)))
DO_READOUT = os.environ.get("KRN_READOUT", "1") == "1"


def build(nc):
    # ---- DRAM parameters (per core) ----
    tok_d = nc.dram_tensor("tok", [T], I32, kind="ExternalInput")
    emb_d = nc.dram_tensor("emb", [V, D], F32, kind="ExternalInput")
    posT_d = nc.dram_tensor("posT", [D, T], F32, kind="ExternalInput")
    dx_d = nc.dram_tensor("dx", [D, K], BF16, kind="ExternalInput")
    dy_d = nc.dram_tensor("dy", [D, K], BF16, kind="ExternalInput")
    e_d = nc.dram_tensor("eh", [K, D], BF16, kind="ExternalInput")
    ro_d = nc.dram_tensor("ro", [D, VS], BF16, kind="ExternalInput")
    cos_d = nc.dram_tensor("cosh", [4, 128, T], BF16, kind="ExternalInput")
    sin_d = nc.dram_tensor("sinh", [4, 128, T], BF16, kind="ExternalInput")
    out_d = nc.dram_tensor("logitsT", [VS, T], F32, kind="ExternalOutput")

    groups = [[0, 1, 2, 3], [4, 5, 6, 7]]

    with tile.TileContext(nc) as tc:
        with (
            nc.allow_low_precision(reason="bf16/f32r rounding is intentional"),
            tc.tile_pool(name="persist", bufs=1) as pp,
            tc.tile_pool(name="work", bufs=1) as wp,
            tc.tile_pool(name="ps", bufs=4, space="PSUM") as psp,
            tc.tile_pool(name="dram", bufs=1, space="DRAM") as dpool,
        ):
            _ctr = [0]

            def _nm(p):
                _ctr[0] += 1
                return f"{p}{_ctr[0]}"

            # pool helpers -- tags control slot sharing
            def xt_t():
                # [128, TH] bf16 x tiles
                return wp.tile([128, TH], BF16, tag="xt", bufs=4, name=_nm("xt_"))

            def rope_t():
                return wp.tile([128, TH], BF16, tag="rope", bufs=2, name=_nm("rp_"))

            def trig_t():
                return wp.tile([128, TH], BF16, tag="trig", bufs=4, name=_nm("tg_"))

            def qtd_t():
                return wp.tile([128, 8, 128], BF16, tag="qtd", bufs=3,
                               name=_nm("qtd_"))

            def f4_t():
                # [128, TH] f32 scratch (pos chunks, E-phase temps)
                return wp.tile([128, TH], F32, tag="f4", bufs=2, name=_nm("f4_"))

            def xr_t():
                # [128, TH] bf16 (x reload, relu result, gated y)
                return wp.tile([128, TH], BF16, tag="xr", bufs=5, name=_nm("xr_"))

            def aT_t():
                return wp.tile([128, ND, TH], BF16, tag="aT", bufs=2,
                               name=_nm("aT_"))

            def sq_t():
                return wp.tile([128, ND, TH], BF16, tag="sq", bufs=2,
                               name=_nm("sq_"))

            def rs_t():
                # [128, TH] f32 broadcast tiles (rstd)
                return wp.tile([128, TH], F32, tag="rs", bufs=2, name=_nm("rs_"))

            def nmb_t():
                # [128, TH] bf16 broadcast tiles (negmean)
                return wp.tile([128, TH], BF16, tag="nmb", bufs=2, name=_nm("nm_"))

            def w8_t():
                # [128, ND, TH] f32 (uT, ye, readout logits)
                return wp.tile([128, ND, TH], F32, tag="w8", bufs=3,
                               name=_nm("w8_"))

            def st_t(dt=F32):
                return wp.tile([1, TH], dt, tag="st", bufs=3, name=_nm("st_"))

            def ps4(shape=None, dt=F32):
                return psp.tile(shape or [128, TH], dt, tag="ps4", name=_nm("ps_"))

            # ---- constants ----
            ident_f = wp.tile([128, 128], F32, tag="idf", bufs=1)
            make_identity(nc, ident_f[:])
            ident_r = pp.tile([128, 128], F32R)
            nc.vector.tensor_copy(ident_r[:], ident_f[:])
            ones_pf = pp.tile([128, 1], F32)
            nc.vector.memset(ones_pf[:], 1.0)
            ones_p = pp.tile([128, 1], F32R)
            nc.vector.tensor_copy(ones_p[:], ones_pf[:])
            ones_pb = pp.tile([128, 1], BF16)
            nc.vector.tensor_copy(ones_pb[:], ones_pf[:])
            ones_cf = pp.tile([1, 128], F32)
            nc.vector.memset(ones_cf[:], 1.0)
            ones_c = pp.tile([1, 128], F32R)
            nc.vector.tensor_copy(ones_c[:], ones_cf[:])
            eps_p = pp.tile([128, 1], F32)
            nc.vector.memset(eps_p[:], EPS)
            eps_1 = pp.tile([1, 1], F32)
            nc.vector.memset(eps_1[:], EPS)
            nc.gpsimd.load_library(library_config.attn)

            # ---- persistent tensors ----
            vT = pp.tile([128, ND, T], F32R)       # v (dT layout), f32 bits
            vpb = pp.tile([128, ND, T], BF16)      # bf16 copy of v (dT layout)
            qT = pp.tile([128, NKT, T], BF16)      # q (kT layout)
            vp_td = pp.tile([128, NT, ND, 128], BF16)  # v+pos (td, xbar tiling)
            m_sb = pp.tile([128, NKT, D], BF16)    # M = q^T (v+pos)  [k, d]
            dx_sb = pp.tile([128, ND, K], BF16)
            nc.sync.dma_start(dx_sb[:], dx_d.ap().rearrange("(c p) k -> p c k", p=128))
            dy_sb = pp.tile([128, ND, K], BF16)
            nc.sync.dma_start(dy_sb[:], dy_d.ap().rearrange("(c p) k -> p c k", p=128))
            e_sb = pp.tile([128, NKT, D], BF16)
            nc.sync.dma_start(e_sb[:], e_d.ap().rearrange("(c p) d -> p c d", p=128))

            # ---- internal DRAM ----
            xspill = dpool.tile([NKT, 128, T], BF16, tag="xspill")
            cc_in = [dpool.tile([ND, 128, TH], F32, tag=f"cci{i}", name=f"cci{i}")
                     for i in range(2)]
            cc_out = [dpool.tile([ND, 128, TH], F32, tag=f"cco{i}", name=f"cco{i}")
                      for i in range(2)]

            def lnstats(s1_ps, s2_fn, n):
                """LN stats chain. Returns (nm_ps, rs_ps) [128, TH] psum bcasts.

                The negmean broadcast is issued as early as possible so
                consumers of the centered value can start before rstd is
                ready. s2_fn() emits the colsum of squares lazily."""
                negmean = st_t(F32R)
                nc.vector.tensor_scalar_mul(negmean[:], s1_ps, -1.0 / n)
                nm_ps = bcast(negmean)
                m2 = st_t()
                nc.vector.tensor_mul(m2[:], negmean[:].bitcast(F32),
                                     negmean[:].bitcast(F32))
                s2 = s2_fn()
                var = st_t()
                nc.vector.scalar_tensor_tensor(
                    out=var[:], in0=s2, scalar=1.0 / n, in1=m2[:],
                    op0=OP.mult, op1=OP.subtract)
                lnv = st_t()
                nc.scalar.activation(lnv[:], var[:], AF.Ln, bias=eps_1[:])
                rstd = st_t(F32R)
                nc.scalar.activation(rstd[:], lnv[:], AF.Exp, scale=-0.5)
                rs_ps = bcast(rstd)
                return nm_ps, rs_ps

            def colsum(mov_fn, ones):
                """s[t] = sum_d mov[d, t] over ND tiles -> [1, TH] psum AP.

                mov_fn(dc, lo, hi) must return a [128, hi-lo] SBUF AP."""
                s = ps4([1, TH])
                for dc in range(ND):
                    for ns in range(NS):
                        nc.tensor.matmul(
                            s[:1, ns * 512:(ns + 1) * 512], ones[:],
                            mov_fn(dc, ns * 512, (ns + 1) * 512),
                            start=(dc == 0), stop=(dc == ND - 1),
                            skip_group_check=True)
                return s[:1, :]

            def bcast(vec):
                """PE rank-1 broadcast of a [1, TH] f32r vector to [128, TH] psum."""
                out = ps4()
                for ns in range(NS):
                    nc.tensor.matmul(out[:, ns * 512:(ns + 1) * 512], ones_c[:],
                                     vec[:, ns * 512:(ns + 1) * 512],
                                     start=True, stop=True)
                return out

            # ============ embedding gather + LN -> v0 -> transpose to vT ============
            idx = pp.tile([128, NT], I32)
            nc.sync.dma_start(idx[:], tok_d.ap().rearrange("(n p) -> p n", p=128))
            for n in range(NT):
                gat = wp.tile([128, D], F32, tag="gat", bufs=2, name=_nm("g_"))
                nc.gpsimd.indirect_dma_start(
                    out=gat[:], out_offset=None, in_=emb_d.ap(),
                    in_offset=bass.IndirectOffsetOnAxis(ap=idx[:, n:n + 1], axis=0),
                )
                stats = wp.tile([128, 6], F32, tag="bst", bufs=2, name=_nm("g_"))
                nc.vector.bn_stats(out=stats[:], in_=gat[:])
                mv = wp.tile([128, 2], F32, tag="bmv", bufs=2, name=_nm("g_"))
                nc.vector.bn_aggr(out=mv[:], in_=stats[:])
                std = wp.tile([128, 1], F32, tag="bsd", bufs=2, name=_nm("g_"))
                nc.scalar.activation(std[:], mv[:, 1:2], AF.Sqrt, bias=eps_p[:])
                rstd = wp.tile([128, 1], F32, tag="brs", bufs=2, name=_nm("g_"))
                nc.vector.reciprocal(rstd[:], std[:])
                v0 = wp.tile([128, D], F32R, tag="gv0", bufs=2, name=_nm("g_"))
                nc.vector.tensor_scalar(
                    out=v0[:], in0=gat[:], scalar1=mv[:, 0:1], scalar2=rstd[:],
                    op0=OP.subtract, op1=OP.mult)
                for dc in range(ND):
                    tp = ps4([128, 128], F32R)
                    nc.tensor.transpose(out=tp[:], in_=v0[:, dc * 128:(dc + 1) * 128],
                                        identity=ident_r[:])
                    nc.vector.tensor_copy(vT[:, dc, n * 128:(n + 1) * 128], tp[:])

            def phaseA(th):
                """v[:, th-half] += pos; refresh vpb + vp_td for that half."""
                t0 = th * TH
                for dc in range(ND):
                    pch = f4_t()
                    nc.sync.dma_start(
                        pch[:], posT_d.ap()[dc * 128:(dc + 1) * 128, t0:t0 + TH])
                    nc.vector.tensor_add(
                        vT[:, dc, t0:t0 + TH],
                        vT[:, dc, t0:t0 + TH].bitcast(F32), pch[:])
                for dc in range(ND):
                    nc.scalar.copy(vpb[:, dc, t0:t0 + TH],
                                   vT[:, dc, t0:t0 + TH].bitcast(F32))
                    nc.sync.dma_start(
                        out=vp_td[:, th * 8:(th + 1) * 8, dc, :],
                        in_=vpb[:, dc, t0:t0 + TH],
                        transpose=True)

            def bphase(th):
                """x[:, th-half] = relu((v+pos) @ Dx); RoPE -> q; spill x."""
                t0 = th * TH
                for i in range(4):
                    cos_t = trig_t()
                    nc.sync.dma_start(cos_t[:], cos_d.ap()[i][:, t0:t0 + TH])
                    sin_t = trig_t()
                    nc.sync.dma_start(sin_t[:], sin_d.ap()[i][:, t0:t0 + TH])
                    xts = {}
                    for ii in (i, i + 4):
                        xt = xt_t()
                        xts[ii] = xt
                        px = ps4()
                        for dc in range(ND):
                            for ns in range(NS):
                                nc.tensor.matmul(
                                    px[:, ns * 512:(ns + 1) * 512],
                                    dx_sb[:, dc, ii * 128:(ii + 1) * 128],
                                    vpb[:, dc, t0 + ns * 512:t0 + (ns + 1) * 512],
                                    start=(dc == 0), stop=(dc == ND - 1))
                        nc.scalar.activation(xt[:], px[:], AF.Relu)
                        nc.sync.dma_start(xspill[ii, :, t0:t0 + TH], xt[:])
                    xi, xj = xts[i], xts[i + 4]
                    m1 = rope_t()
                    nc.vector.tensor_mul(m1[:], xi[:], cos_t[:])
                    m2 = rope_t()
                    nc.vector.tensor_mul(m2[:], xj[:], sin_t[:])
                    nc.vector.tensor_sub(qT[:, i, t0:t0 + TH], m1[:], m2[:])
                    m3 = rope_t()
                    nc.vector.tensor_mul(m3[:], xj[:], cos_t[:])
                    m4 = rope_t()
                    nc.vector.tensor_mul(m4[:], xi[:], sin_t[:])
                    nc.vector.tensor_add(qT[:, i + 4, t0:t0 + TH], m3[:], m4[:])

            # ================================ layers ================================
            for layer in range(N_LAYERS):
                if layer == 0:
                    for th in range(2):
                        phaseA(th)
                        bphase(th)

                # ---- C1: M[k,d] = sum_t q[t,k] (v+pos)[t,d] via xbar q_td ----
                pms = [ps4([128, 4, 256]), ps4([128, 4, 256])]
                for kc in (0, 4, 1, 5, 2, 6, 3, 7):
                    pm = pms[kc // 4][:, kc % 4, :]
                    for th in range(2):
                        qtd = qtd_t()
                        nc.sync.dma_start(
                            out=qtd[:],
                            in_=qT[:, kc, th * TH:(th + 1) * TH],
                            transpose=True)
                        for n in range(8):
                            nc.tensor.matmul(
                                pm, qtd[:, n, :],
                                vp_td[:, th * 8 + n, :, :],
                                start=(th == 0 and n == 0),
                                stop=(th == 1 and n == 7),
                                skip_group_check=True)
                    nc.vector.tensor_copy(m_sb[:, kc, :], pm)

                # ---- C2 + LN(a) + D per t-half ----
                for th in range(2):
                    t0 = th * TH
                    # C2: aT[d, t-half] = sum_k M[k,d] q[k,t]
                    aT = aT_t()
                    sq = sq_t()
                    for dc in range(ND):
                        pa = ps4()
                        for kc in range(NKT):
                            for ns in range(NS):
                                nc.tensor.matmul(
                                    pa[:, ns * 512:(ns + 1) * 512],
                                    m_sb[:, kc, dc * 128:(dc + 1) * 128],
                                    qT[:, kc, t0 + ns * 512:t0 + (ns + 1) * 512],
                                    start=(kc == 0), stop=(kc == NKT - 1))
                        nc.scalar.copy(aT[:, dc], pa[:])
                        nc.scalar.activation(sq[:, dc], pa[:], AF.Square)
                    # LN(a) stats; rstd folded into the y@E output scale
                    s1 = colsum(lambda dc, lo, hi: aT[:, dc, lo:hi], ones_pb)
                    nm_ps, rs_ps = lnstats(
                        s1, lambda: colsum(
                            lambda dc, lo, hi: sq[:, dc, lo:hi], ones_pb), D)
                    nm_s = nmb_t()
                    nc.scalar.copy(nm_s[:], nm_ps[:])
                    rs_s = rs_t()
                    nc.scalar.copy(rs_s[:], rs_ps[:])
                    for dc in range(ND):
                        nc.vector.tensor_add(aT[:, dc], aT[:, dc], nm_s[:])

                    # D: y_i = relu((a-mu) @ Dy_i) * x_i ; pye = sum_i E_i^T y_i
                    pyes = [ps4(), ps4()]
                    for i in range(NKT):
                        py = ps4()
                        for dc in range(ND):
                            for ns in range(NS):
                                nc.tensor.matmul(
                                    py[:, ns * 512:(ns + 1) * 512],
                                    dy_sb[:, dc, i * 128:(i + 1) * 128],
                                    aT[:, dc, ns * 512:(ns + 1) * 512],
                                    start=(dc == 0), stop=(dc == ND - 1))
                        rl = xr_t()
                        nc.scalar.activation(rl[:], py[:], AF.Relu)
                        xr = xr_t()
                        nc.sync.dma_start(xr[:], xspill[i, :, t0:t0 + TH])
                        yt = xr_t()
                        nc.vector.tensor_mul(yt[:], rl[:], xr[:])
                        for dc in range(ND):
                            for ns in range(NS):
                                nc.tensor.matmul(
                                    pyes[dc][:, ns * 512:(ns + 1) * 512],
                                    e_sb[:, i, dc * 128:(dc + 1) * 128],
                                    yt[:, ns * 512:(ns + 1) * 512],
                                    start=(i == 0), stop=(i == NKT - 1),
                                    skip_group_check=True)
                    ye = w8_t()
                    for dc in range(ND):
                        nc.vector.tensor_mul(ye[:, dc], pyes[dc][:], rs_s[:])
                        nc.sync.dma_start(cc_in[th][:][dc], ye[:, dc])
                    nc.gpsimd.collective_compute(
                        "AllReduce", OP.add, replica_groups=groups,
                        ins=[cc_in[th][:].opt()], outs=[cc_out[th][:].opt()])

                # ---- E: u = ln(sum); w = vp + u; v = ln(w); then next-B ----
                for th in range(2):
                    t0 = th * TH
                    uT = w8_t()
                    squ = sq_t()
                    for dc in range(ND):
                        nc.sync.dma_start(uT[:, dc], cc_out[th][:][dc])
                        nc.scalar.activation(squ[:, dc], uT[:, dc], AF.Square)
                    s1 = colsum(lambda dc, lo, hi: uT[:, dc, lo:hi].bitcast(F32R),
                                ones_p)
                    nm_u, rs_u = lnstats(
                        s1, lambda: colsum(
                            lambda dc, lo, hi: squ[:, dc, lo:hi], ones_pb), D)
                    for dc in range(ND):
                        t1 = f4_t()
                        nc.vector.tensor_add(t1[:], uT[:, dc], nm_u[:])
                        nc.vector.tensor_mul(t1[:], t1[:], rs_u[:])
                        nc.vector.tensor_add(vT[:, dc, t0:t0 + TH],
                                             vT[:, dc, t0:t0 + TH].bitcast(F32),
                                             t1[:])
                    sqw = sq_t()
                    for dc in range(ND):
                        nc.scalar.activation(sqw[:, dc],
                                             vT[:, dc, t0:t0 + TH].bitcast(F32),
                                             AF.Square)
                    s1 = colsum(lambda dc, lo, hi: vT[:, dc, t0 + lo:t0 + hi],
                                ones_p)
                    nm_w, rs_w = lnstats(
                        s1, lambda: colsum(
                            lambda dc, lo, hi: sqw[:, dc, lo:hi], ones_pb), D)
                    for dc in range(ND):
                        nc.vector.tensor_add(vT[:, dc, t0:t0 + TH],
                                             vT[:, dc, t0:t0 + TH].bitcast(F32),
                                             nm_w[:])
                        nc.vector.tensor_mul(vT[:, dc, t0:t0 + TH],
                                             vT[:, dc, t0:t0 + TH].bitcast(F32),
                                             rs_w[:])
                    if layer < N_LAYERS - 1:
                        phaseA(th)
                        bphase(th)
                    else:
                        # final vpb refresh so the bf16 readout sees final v
                        for dc in range(ND):
                            nc.scalar.copy(vpb[:, dc, t0:t0 + TH],
                                           vT[:, dc, t0:t0 + TH].bitcast(F32))

            # ============= readout: logitsT = (v @ readout)^T, V-sharded =============
            if DO_READOUT:
                ro_r = ro_d.ap().rearrange("(c p) v -> p c v", p=128)
                nvb = (VS + 127) // 128
                for vb in range(nvb):
                    m = min(128, VS - vb * 128)
                    ro_sb = wp.tile([128, ND, 128], BF16, tag="ro",
                                    bufs=3, name=_nm("ro_"))
                    nc.sync.dma_start(ro_sb[:, :, :m], ro_r[:, :, vb * 128:vb * 128 + m])
                    lo = w8_t()
                    for th in range(2):
                        pl = ps4()
                        for dc in range(ND):
                            for ns in range(NS):
                                nc.tensor.matmul(
                                    pl[:m, ns * 512:(ns + 1) * 512],
                                    ro_sb[:, dc, :m],
                                    vpb[:, dc, th * TH + ns * 512:
                                        th * TH + (ns + 1) * 512],
                                    start=(dc == 0), stop=(dc == ND - 1))
                        if (vb + th) % 2 == 0:
                            nc.scalar.copy(lo[:m, th], pl[:m])
                        else:
                            nc.vector.tensor_copy(lo[:m, th], pl[:m])
                    nc.sync.dma_start(
                        out_d.ap()[vb * 128:vb * 128 + m, :], lo[:m])

    nc.compile()
    return nc


_NC_CACHE = None


def _get_nc():
    global _NC_CACHE
    if _NC_CACHE is None:
        nc = bacc.Bacc("TRN2", target_bir_lowering=False, debug=False, num_devices=8)
        _NC_CACHE = build(nc)
    return _NC_CACHE


def _rope_tables():
    # match the jax reference: float32 angle computation
    inv_freq = (1.0 / (10000.0 ** (np.arange(0, K, 2, dtype=np.float32)
                                   / np.float32(K)))).astype(np.float32)
    t = np.arange(T, dtype=np.float32)
    freqs = (t[:, None] * inv_freq[None, :]).astype(np.float32)  # [T, K/2]
    cos = np.cos(freqs).astype(ml_dtypes.bfloat16)
    sin = np.sin(freqs).astype(ml_dtypes.bfloat16)
    cosT = np.ascontiguousarray(cos.T).reshape(4, 128, T)
    sinT = np.ascontiguousarray(sin.T).reshape(4, 128, T)
    return cosT, sinT


def kernel(input_, emb, pos, Dx, Dy, E, readout):
    input_ = np.asarray(input_)
    emb = np.ascontiguousarray(np.asarray(emb, dtype=np.float32))
    pos = np.asarray(pos, dtype=np.float32)
    Dx = np.asarray(Dx, dtype=np.float32)
    Dy = np.asarray(Dy, dtype=np.float32)
    E = np.asarray(E, dtype=np.float32)
    readout = np.asarray(readout, dtype=np.float32)

    nc = _get_nc()
    cosT, sinT = _rope_tables()
    posT = np.ascontiguousarray(pos.T)

    in_maps = []
    for c in range(8):
        b, h = divmod(c, 4)
        in_maps.append({
            "tok": np.ascontiguousarray(input_[b].astype(np.int32)),
            "emb": emb,
            "posT": posT,
            "dx": np.ascontiguousarray(Dx[h]).astype(ml_dtypes.bfloat16),
            "dy": np.ascontiguousarray(Dy[h]).astype(ml_dtypes.bfloat16),
            "eh": np.ascontiguousarray(E[h * K:(h + 1) * K]).astype(
                ml_dtypes.bfloat16),
            "ro": np.ascontiguousarray(readout[:, h * VS:(h + 1) * VS]).astype(
                ml_dtypes.bfloat16),
            "cosh": cosT,
            "sinh": sinT,
        })
    trace = os.environ.get("KRN_TRACE", "0") == "1"
    res = run_bass_kernel_spmd(nc, in_maps, list(range(8)), trace=trace)
    out = np.empty((B, T, V), dtype=np.float32)
    for c in range(8):
        b, h = divmod(c, 4)
        out[b, :, h * VS:(h + 1) * VS] = res.results[c]["logitsT"].T
    kernel._last_results = res
    return out
